# revision 36
# baseline (speedup 1.0000x reference)
"""Trainium2 Bass kernel for CPELayer_ResAG (concept-routed LoRA edit layer).

Computation (per token t with concept c = concept_idx[t]):
    down = edit_direction[t] @ lora_down[c]          # [768]@[768,4] -> [4]
    up   = down @ lora_up[c]                         # [4]@[4,1280]  -> [1280]
    out  = x[t] @ W.T + b_lin + 0.25 * up

Strategy: data-parallel over batch across 8 cores (616 tokens/core).
The routed LoRA is computed densely for ALL concepts (A.T = ld.T @ ed.T,
~6% extra PE work), masked on-device with a one-hot built by DVE is_equal,
and contracted back with lora_up, accumulating into the same PSUM group as
the org matmul.

v7 schedule:
 - org matmul runs fully in fp8-e3m4 (5 significant bits): x.T*2 and
   W.T*64, so the PSUM holds 128*out; the whole scale chain (masks at 1.0,
   bias row *1024) carries the 128, and the host divides the bf16 output
   by 128 (exact).  Measured end-to-end absmax err 0.109 on output scale
   7.54 (rel 1.44e-2) vs the 2e-2 gate, on the fixed harness seed.
 - input ring order = need order with the measured (serial-completion +
   skew) DMA model: cvidx, xw0, xw12, led, lu8, xw34, xw5.  The A.T
   chain runs MID-wave1 (its data lands after xw12), where the PE queue
   is data-rich, instead of gating the stream start.
 - the critical path is xw12's DMA completion: everything before it is
   bridged with warmup matmuls sized so wave1 k1 starts the moment the
   data lands; warmup also covers the ~3.4us HAM window so all real
   matmuls run at 2.4GHz.

Host-side work is layout/dtype only; every FLOP of the reference runs on
device.
"""

import sys
import types

import numpy as np

import concourse.mybir as mybir
import concourse.tile as tile
from concourse import bacc
from concourse.bass_utils import run_bass_kernel_spmd

# If BASS_TRACE is set in the environment, run_bass_kernel_spmd imports
# antenv.axon_hooks, which some containers lack; stub it (None hook ->
# tracing is skipped gracefully, execution unaffected).
try:
    import antenv.axon_hooks  # noqa: F401
except ImportError:
    _m = types.ModuleType("antenv.axon_hooks")
    _m.get_axon_ntff_profile_hook = lambda: None
    _m.set_axon_ntff_profile_hook = lambda h: None
    sys.modules["antenv.axon_hooks"] = _m

# Problem shapes (hardcoded per spec nn_CPELayer_ResAG_19335942766951)
N_CORES = 8
B, T, DIN, DOUT = 64, 77, 768, 1280
N_CONCEPTS, RANK = 50, 4
BPC = B // N_CORES          # batches per core = 8
TOK = BPC * T               # tokens per core = 616
NJ = N_CONCEPTS * RANK      # 200 flattened (concept, rank) rows
P = 128
KD = DIN // P               # 6 k-tiles of the d_in contraction
NH = 308                    # half of TOK for the A.T psum tiles
# led layout, split by ed-half so the first two A.T chains (nh=0) only
# need the first DMA: block A [128, 6, 528] holds ldT*16 at cols 0:200 and
# ed half0 at cols 208:516; block B [128, 6, 336] holds ed half1 at 0:308.
# All DoubleRow bases and subtile byte-steps are 16-aligned (208/528/336).
ED_OFF = 208
LEDA_W = 528
LEDB_W = 336
# xw k-tile layout (fp8-e3m4): [0:616] = x.T*2; [624:1904] = W.T*64.
# WOFF=624 keeps every wave matmul operand base 16-byte aligned.
XW_W = 1904
WOFF = 624
T_EDGES = [0, 128, 256, 384, 512, 616]
N_CHUNKS = [(0, 512), (512, 512), (1024, 256)]
N_WARM = 12                 # warmup matmuls (256 cols each, ~213ns cold)
N_WARM2 = 0                 # bridge fillers between wave1-k0 and k1 data

_cache = {}


def _build_bass():
    nc = bacc.Bacc("TRN2", target_bir_lowering=False, debug=False,
                   num_devices=N_CORES)
    f32 = mybir.dt.float32
    bf16 = mybir.dt.bfloat16
    f8 = mybir.dt.float8e4
    f8e3 = mybir.dt.float8e3
    u8 = mybir.dt.uint8
    DR = mybir.MatmulPerfMode.DoubleRow

    # All big inputs are partition-major [128, W] in DRAM (one fat row per
    # partition -> one large DMA descriptor per partition line; ~2-4KB rows
    # hit the per-packet sweet spot of the DMA engines).
    cvidx_d = nc.dram_tensor("cvidx", [P, 8 + TOK], u8,
                             kind="ExternalInput").ap()
    xw0_d = nc.dram_tensor("xw0", [P, XW_W], f8e3,
                           kind="ExternalInput").ap()
    xw1_d = nc.dram_tensor("xw1", [P, XW_W], f8e3,
                           kind="ExternalInput").ap()
    xw2_d = nc.dram_tensor("xw2", [P, XW_W], f8e3,
                           kind="ExternalInput").ap()
    leda_d = nc.dram_tensor("leda", [P, KD * LEDA_W], f8,
                            kind="ExternalInput").ap()
    ledb_d = nc.dram_tensor("ledb", [P, KD * LEDB_W], f8,
                            kind="ExternalInput").ap()
    lu_d = nc.dram_tensor("lu8", [P, 2 * DOUT], f8, kind="ExternalInput").ap()
    xw34_d = nc.dram_tensor("xw34", [P, 2 * XW_W], f8e3,
                            kind="ExternalInput").ap()
    xw5_d = nc.dram_tensor("xw5", [P, XW_W], f8e3,
                           kind="ExternalInput").ap()
    out_d = nc.dram_tensor("out", [TOK, DOUT], bf16, kind="ExternalOutput").ap()

    with tile.TileContext(nc) as tc:
        with (
            tc.tile_pool(name="consts", bufs=1) as consts,
            tc.tile_pool(name="outsb", bufs=5) as outsb,
        ):
            # Input DMAs: all doorbells on the sync queue in strict
            # need-order (DMA-ring work order = enqueue order; a transfer's
            # completion ~= serial-prefix-bytes/~265GBps + up to ~1.5us of
            # engine skew).
            # The 16 DMA engines serve ALL enqueued transfers round-robin
            # at ~21.5GB/s each, so an early doorbell steals bandwidth from
            # every in-flight transfer.  Only the critical prefix (cvidx,
            # xw0, xw12 - the wave1 k1 gate) rings immediately; led rings
            # from scalar behind burn-copies that write into the led tiles
            # (WAW ordering the scheduler cannot hoist), and lu8/xw345
            # ring from gpsimd behind memset burns the same way.
            # Per-k-tile transfers: wave1's group pipeline consumes k0 and
            # k1 for the first 6 psum groups while k2 is still streaming,
            # so the dense-stream onset is the xw2 completion (~10.9us).
            # xw0 rings from scalar (earliest body entry, empty Q10 ring)
            # so the k0 groups start ~0.5us sooner.
            xw0 = consts.tile([P, 1, XW_W], f8e3, tag="xw0")
            nc.scalar.dma_start(xw0[:],
                                xw0_d.rearrange("p (k c) -> p k c", c=XW_W))
            xw1 = consts.tile([P, 1, XW_W], f8e3, tag="xw1")
            nc.sync.dma_start(xw1[:],
                              xw1_d.rearrange("p (k c) -> p k c", c=XW_W))
            xw2 = consts.tile([P, 1, XW_W], f8e3, tag="xw2")
            nc.sync.dma_start(xw2[:],
                              xw2_d.rearrange("p (k c) -> p k c", c=XW_W))
            cvidx = consts.tile([P, 8 + TOK], u8, tag="cvidx")
            nc.sync.dma_start(cvidx[:], cvidx_d[:, :])
            cvals = cvidx[:, 0:8].bitcast(f32)
            idx_bc = cvidx[:, 8:8 + TOK]

            # Hoist the scalar engine's one-time ACT_TABLE_LOAD into the
            # boot shadow (it otherwise fires lazily right before the first
            # wave1 copy, delaying it by ~1.5us).
            scratch = consts.tile([1, 8], f32, tag="scratch")
            nc.vector.memset(scratch[:], 0.0)
            nc.scalar.copy(out=scratch[:, 0:4], in_=scratch[:, 4:8])

            zeros = consts.tile([P, 1024], f32, tag="zeros")
            nc.vector.memset(zeros[:], 0.0)

            led_a = consts.tile([P, KD, LEDA_W], f8, tag="led_a")
            led_b = consts.tile([P, KD, LEDB_W], f8, tag="led_b")
            # ~1.4us scalar burn, then the led doorbells (~10.5us, as the
            # critical prefix finishes).
            nc.scalar.copy(out=led_a[:, 0, 0:512], in_=zeros[:, 0:512])
            nc.scalar.copy(out=led_a[:, 1, 0:512], in_=zeros[:, 0:512])
            nc.scalar.dma_start(led_a[:],
                                leda_d.rearrange("p (k c) -> p k c",
                                                 c=LEDA_W))
            nc.scalar.copy(out=led_b[:, 0, 0:336], in_=zeros[:, 0:336])
            nc.scalar.dma_start(led_b[:],
                                ledb_d.rearrange("p (k c) -> p k c",
                                                 c=LEDB_W))

            # PE warmup: dummy matmuls during the boot/DMA shadow covering
            # the full ~3.4us HAM activity window (cold matmuls run at
            # 1.2GHz) AND bridging the PE queue to the xw12 arrival.
            warm = consts.tile([P, 256], bf16, tag="warm")
            nc.gpsimd.memset(warm[:], 0.0)

            lu8 = consts.tile([P, 2, DOUT], f8, tag="lu8")
            xw34 = consts.tile([P, 2, XW_W], f8e3, tag="xw34")
            xw5 = consts.tile([P, 1, XW_W], f8e3, tag="xw5")

            def xt(kk, tsl):
                t_, o = ((xw0, 0) if kk < 1 else
                         (xw1, 1) if kk < 2 else
                         (xw2, 2) if kk < 3 else
                         (xw34, 3) if kk < 5 else (xw5, 5))
                return t_[:, kk - o, tsl]

            def wt(kk, nsl):
                t_, o = ((xw0, 0) if kk < 1 else
                         (xw1, 1) if kk < 2 else
                         (xw2, 2) if kk < 3 else
                         (xw34, 3) if kk < 5 else (xw5, 5))
                return t_[:, kk - o,
                          slice(WOFF + nsl.start, WOFF + nsl.stop)]

            # One-hot masks: mask[p, t] = (concept_idx[t] == cvals[p, jc]).
            # masks on vector (gpsimd runs this op ~20x slower: 9.5us/mask).
            masks = []
            for jc in range(2):
                m = consts.tile([P, TOK], f32, tag=f"mask{jc}")
                nc.vector.tensor_scalar(
                    m[:], idx_bc[:], cvals[:, jc:jc + 1], 1.0,
                    mybir.AluOpType.is_equal, mybir.AluOpType.mult)
                masks.append(m)

            # MT8[(c,r) rows as [part, subtile], t]: routed "down"
            # activations (16*down) in fp8, consumed by the DoubleRow
            # up-matmul.  Rows 200..255 (partitions 72..127 of subtile 1)
            # are zero except the ones row at 224 (partition 96) that
            # contracts with the *1024 bias row of lu8.
            MT8 = consts.tile([P, 2, 640], f8, tag="MT8")
            nc.gpsimd.memset(MT8[64:P, 1, :], 0.0)
            nc.gpsimd.memset(MT8[96:97, 1, :], 0.125)

            # gpsimd burn chain (~1ns/byte-per-partition): doorbells for
            # the wave2 operands fire ~13-15us, after the led stream.
            nc.gpsimd.memset(lu8[:, 0, :], 0.0)
            nc.gpsimd.memset(lu8[:, 1, :], 0.0)
            nc.gpsimd.dma_start(lu8[:],
                                lu_d.rearrange("p (j o) -> p j o", o=DOUT))
            nc.gpsimd.memset(xw34[:, 0, :], 0.0)
            nc.gpsimd.dma_start(xw34[:],
                                xw34_d.rearrange("p (k c) -> p k c", c=XW_W))
            nc.gpsimd.memset(xw5[:, 0, 0:1000], 0.0)
            nc.gpsimd.dma_start(xw5[:],
                                xw5_d.rearrange("p (k c) -> p k c", c=XW_W))

            with (
                tc.tile_pool(name="at_ps", bufs=2, space="PSUM") as at_pool,
                tc.tile_pool(name="out_ps", bufs=6, space="PSUM") as out_pool,
            ):
                # warm matmuls write an out_pool tile; its buffer recycles
                # into a wave1 group long after the warmup retires.
                wps = out_pool.tile([P, 512], f32, tag="ops")
                for i in range(N_WARM):
                    nc.tensor.matmul(wps[:, 0:256], warm[:, 0:P], warm[:, :],
                                     start=(i == 0), stop=(i == N_WARM - 1))

                # A.T[(c,r), t] = (16*lora_down_flat).T @ ed.T for all
                # concepts, 3 DoubleRow matmuls per chunk (contraction row =
                # sub*128 + p).  The led data lands mid-wave1; the scheduler
                # interleaves these chains into the wave1 stream.
                for nh in range(2):
                    for jc in range(2):
                        jp = P if jc == 0 else NJ - P  # 128, 72
                        jsl = slice(jc * P, jc * P + jp)
                        nsl = slice(nh * NH, (nh + 1) * NH)
                        at = at_pool.tile([P, NH], f32, tag="at")
                        for k in range(KD // 2):
                            if nh == 0:
                                rhs = led_a[:, 2 * k:2 * k + 2,
                                            ED_OFF:ED_OFF + NH]
                            else:
                                rhs = led_b[:, 2 * k:2 * k + 2, 0:NH]
                            nc.tensor.matmul(
                                at[:jp, :],
                                led_a[:, 2 * k:2 * k + 2, jsl], rhs,
                                start=(k == 0), stop=(k == KD // 2 - 1),
                                perf_mode=DR)
                        nc.vector.tensor_tensor(
                            MT8[:jp, jc, nsl], at[:jp, :],
                            masks[jc][:jp, nsl], mybir.AluOpType.mult)

                if N_WARM2:
                    # Bridge fillers between the wave1-k0 groups and the
                    # xw12 arrival.
                    wps2 = out_pool.tile([P, 512], f32, tag="ops")
                    for i in range(N_WARM2):
                        nc.tensor.matmul(wps2[:, 0:256], warm[:, 0:P],
                                         warm[:, :], start=(i == 0),
                                         stop=(i == N_WARM2 - 1))

                # Main accumulation: wave1 [org k0..k2] per (t, n) PSUM
                # group -> copy to bf16 osb on scalar; wave2 [upDR, org
                # k3..k5] (DR must open its group: a fp8->DR transition
                # mid-group wedges the PE) -> vector add into osb -> bf16
                # output DMA.
                KA = 3
                osbs = []
                for ti in range(len(T_EDGES) - 1):
                    t0, t1 = T_EDGES[ti], T_EDGES[ti + 1]
                    tw = t1 - t0
                    tsl = slice(t0, t1)
                    osb = outsb.tile([P, DOUT], bf16, tag="osb")
                    osbs.append(osb)
                    for (n0, nw) in N_CHUNKS:
                        nsl = slice(n0, n0 + nw)
                        ps = out_pool.tile([P, 512], f32, tag="ops")
                        for k in range(KA):
                            nc.tensor.matmul(
                                ps[:tw, :nw], xt(k, tsl), wt(k, nsl),
                                start=(k == 0), stop=(k == KA - 1))
                        nc.scalar.copy(out=osb[:tw, nsl], in_=ps[:tw, :nw])
                for ti in range(len(T_EDGES) - 1):
                    t0, t1 = T_EDGES[ti], T_EDGES[ti + 1]
                    tw = t1 - t0
                    tsl = slice(t0, t1)
                    osb = osbs[ti]
                    for (n0, nw) in N_CHUNKS:
                        nsl = slice(n0, n0 + nw)
                        ps = out_pool.tile([P, 512], f32, tag="ops")
                        nc.tensor.matmul(
                            ps[:tw, :nw], MT8[:, 0:2, tsl],
                            lu8[:, 0:2, nsl],
                            start=True, stop=False, perf_mode=DR)
                        for i, k in enumerate(range(KA, KD)):
                            nc.tensor.matmul(
                                ps[:tw, :nw], xt(k, tsl), wt(k, nsl),
                                start=False, stop=(i == KD - KA - 1))
                        nc.vector.tensor_tensor(
                            osb[:tw, nsl], ps[:tw, :nw], osb[:tw, nsl],
                            mybir.AluOpType.add)
                        if ti == len(T_EDGES) - 2:
                            # last t-tile: ship each n-chunk as soon as its
                            # add lands, on the (by now idle) sync input
                            # queue / the gpsimd output queue; scalar's
                            # doorbells measure ~1.5us - avoid it here.
                            eng = (nc.sync, nc.sync, nc.gpsimd)[n0 // 512]
                            eng.dma_start(out_d[tsl, nsl], osb[:tw, nsl])
                    if ti == len(T_EDGES) - 3:
                        # second-to-last t-tile rides the scalar queue
                        # (idle after the wave1 copies; its slow doorbell
                        # is off the critical path here), keeping sync free
                        # to ship the last tile's chunks immediately.
                        nc.scalar.dma_start(out_d[tsl, :], osb[:tw, :])
                    elif ti != len(T_EDGES) - 2:
                        nc.gpsimd.dma_start(out_d[tsl, :], osb[:tw, :])

    nc.compile()
    return nc


def get_bass():
    if "v7" not in _cache:
        _cache["v7"] = _build_bass()
    return _cache["v7"]


def make_in_maps(x, edit_direction, concept_idx, lora_down, lora_up, W, b_lin):
    """Host-side sharding + layout/dtype prep (no reference FLOPs)."""
    f8 = mybir.dt.np(mybir.dt.float8e4)
    f8e3 = mybir.dt.np(mybir.dt.float8e3)
    x = np.asarray(x, dtype=np.float32)
    ed = np.asarray(edit_direction, dtype=np.float32)
    idx = np.asarray(concept_idx)
    ld = np.asarray(lora_down, dtype=np.float32)
    lup = np.asarray(lora_up, dtype=np.float32)
    W = np.asarray(W, dtype=np.float32)
    b = np.asarray(b_lin, dtype=np.float32)

    def pmajor(a, ksub):
        """[ksub*128, W] row-major -> [128, ksub*W] partition-major."""
        w = a.shape[1]
        return np.ascontiguousarray(
            a.reshape(ksub, P, w).transpose(1, 0, 2).reshape(P, ksub * w))

    ldT = ld.transpose(1, 0, 2).reshape(DIN, NJ)                # [768, 200]
    lu8 = np.zeros((2 * P, DOUT), dtype=np.float32)
    lu8[:NJ] = lup.reshape(NJ, DOUT) * 2.0   # x8 (range) x0.25 (alpha/rank)
    lu8[P + 96] = b * 1024.0                 # bias row (*128 psum, ones=1/8)
    lu8 = pmajor(lu8.astype(f8), 2)                             # [128, 2560]
    cv = np.full(2 * P, -1.0, dtype=np.float32)
    cv[:NJ] = np.arange(NJ, dtype=np.float32) // RANK
    cvals = np.ascontiguousarray(cv.reshape(2, P).T)            # [128, 2]
    # Whole org matmul in fp8-e3m4: (2x).T @ (64 W).T accumulates 128*out
    # in PSUM; the host divides the bf16 output by 128 (exact).
    WT8 = np.clip(W.T * 64.0, -15.5, 15.5).astype(f8e3)         # [768, 1280]

    in_maps = []
    for c in range(N_CORES):
        sl = slice(c * BPC, (c + 1) * BPC)
        xs = x[sl].reshape(TOK, DIN)
        eds = ed[sl].reshape(TOK, DIN)
        idxu = idx[sl].reshape(TOK).astype(np.uint8)
        leda = np.zeros((DIN, LEDA_W), dtype=f8)
        leda[:, :NJ] = (ldT * 16.0).astype(f8)
        edT8 = eds.T.astype(f8)
        leda[:, ED_OFF:ED_OFF + NH] = edT8[:, :NH]
        ledb = np.zeros((DIN, LEDB_W), dtype=f8)
        ledb[:, :NH] = edT8[:, NH:]
        cvidx = np.empty((P, 8 + TOK), dtype=np.uint8)
        cvidx[:, 0:8] = cvals.view(np.uint8)
        cvidx[:, 8:] = np.broadcast_to(idxu.reshape(1, TOK), (P, TOK))
        xw = np.zeros((DIN, XW_W), dtype=f8e3)
        xw[:, :TOK] = np.clip(xs.T * 2.0, -15.5, 15.5).astype(f8e3)
        xw[:, WOFF:] = WT8
        xw = pmajor(xw, KD)                                     # [128, 6*XW_W]
        in_maps.append({
            "cvidx": cvidx,
            "xw0": np.ascontiguousarray(xw[:, 0:XW_W]),
            "xw1": np.ascontiguousarray(xw[:, XW_W:2 * XW_W]),
            "xw2": np.ascontiguousarray(xw[:, 2 * XW_W:3 * XW_W]),
            "leda": pmajor(leda, KD),
            "ledb": pmajor(ledb, KD),
            "lu8": lu8,
            "xw34": np.ascontiguousarray(xw[:, 3 * XW_W:5 * XW_W]),
            "xw5": np.ascontiguousarray(xw[:, 5 * XW_W:]),
        })
    return in_maps


def kernel(x, edit_direction, concept_idx, lora_down, lora_up, W, b_lin,
           _trace=False):
    nc = get_bass()
    in_maps = make_in_maps(x, edit_direction, concept_idx, lora_down, lora_up,
                           W, b_lin)
    res = run_bass_kernel_spmd(nc, in_maps, core_ids=list(range(N_CORES)),
                               trace=_trace)
    out = np.concatenate([np.asarray(r["out"], dtype=np.float32)
                          for r in res.results], axis=0) * (1.0 / 128.0)
    out = out.reshape(B, T, DOUT)
    if _trace:
        kernel.last_results = res
    return out


# revision 37
# speedup vs baseline: 1.0300x; 1.0300x over previous
"""Trainium2 Bass kernel for CPELayer_ResAG (concept-routed LoRA edit layer).

Computation (per token t with concept c = concept_idx[t]):
    down = edit_direction[t] @ lora_down[c]          # [768]@[768,4] -> [4]
    up   = down @ lora_up[c]                         # [4]@[4,1280]  -> [1280]
    out  = x[t] @ W.T + b_lin + 0.25 * up

Strategy: data-parallel over batch across 8 cores (616 tokens/core).
The routed LoRA is computed densely for ALL concepts (A.T = ld.T @ ed.T,
~6% extra PE work), masked on-device with a one-hot built by DVE is_equal,
and contracted back with lora_up, accumulating into the same PSUM group as
the org matmul.

v7 schedule:
 - org matmul runs fully in fp8-e3m4 (5 significant bits): x.T*2 and
   W.T*64, so the PSUM holds 128*out; the whole scale chain (masks at 1.0,
   bias row *1024) carries the 128, and the host divides the bf16 output
   by 128 (exact).  Measured end-to-end absmax err 0.109 on output scale
   7.54 (rel 1.44e-2) vs the 2e-2 gate, on the fixed harness seed.
 - input ring order = need order with the measured (serial-completion +
   skew) DMA model: cvidx, xw0, xw12, led, lu8, xw34, xw5.  The A.T
   chain runs MID-wave1 (its data lands after xw12), where the PE queue
   is data-rich, instead of gating the stream start.
 - the critical path is xw12's DMA completion: everything before it is
   bridged with warmup matmuls sized so wave1 k1 starts the moment the
   data lands; warmup also covers the ~3.4us HAM window so all real
   matmuls run at 2.4GHz.

Host-side work is layout/dtype only; every FLOP of the reference runs on
device.
"""

import sys
import types

import numpy as np

import concourse.mybir as mybir
import concourse.tile as tile
from concourse import bacc
from concourse.bass_utils import run_bass_kernel_spmd

# If BASS_TRACE is set in the environment, run_bass_kernel_spmd imports
# antenv.axon_hooks, which some containers lack; stub it (None hook ->
# tracing is skipped gracefully, execution unaffected).
try:
    import antenv.axon_hooks  # noqa: F401
except ImportError:
    _m = types.ModuleType("antenv.axon_hooks")
    _m.get_axon_ntff_profile_hook = lambda: None
    _m.set_axon_ntff_profile_hook = lambda h: None
    sys.modules["antenv.axon_hooks"] = _m

# Problem shapes (hardcoded per spec nn_CPELayer_ResAG_19335942766951)
N_CORES = 8
B, T, DIN, DOUT = 64, 77, 768, 1280
N_CONCEPTS, RANK = 50, 4
BPC = B // N_CORES          # batches per core = 8
TOK = BPC * T               # tokens per core = 616
NJ = N_CONCEPTS * RANK      # 200 flattened (concept, rank) rows
P = 128
KD = DIN // P               # 6 k-tiles of the d_in contraction
NH = 308                    # half of TOK for the A.T psum tiles
# led layout, split by ed-half so the first two A.T chains (nh=0) only
# need the first DMA: block A [128, 6, 528] holds ldT*16 at cols 0:200 and
# ed half0 at cols 208:516; block B [128, 6, 336] holds ed half1 at 0:308.
# All DoubleRow bases and subtile byte-steps are 16-aligned (208/528/336).
ED_OFF = 208
LEDA_W = 528
LEDB_W = 336
# xw k-tile layout (fp8-e3m4): [0:616] = x.T*2; [624:1904] = W.T*64.
# WOFF=624 keeps every wave matmul operand base 16-byte aligned.
XW_W = 1904
WOFF = 624
T_EDGES = [0, 128, 256, 384, 512, 616]
N_CHUNKS = [(0, 512), (512, 512), (1024, 256)]
N_WARM = 16                 # warmup matmuls (256 cols each, ~213ns cold)
N_WARM2 = 0                 # bridge fillers between wave1-k0 and k1 data

_cache = {}


def _build_bass():
    nc = bacc.Bacc("TRN2", target_bir_lowering=False, debug=False,
                   num_devices=N_CORES)
    f32 = mybir.dt.float32
    bf16 = mybir.dt.bfloat16
    f8 = mybir.dt.float8e4
    f8e3 = mybir.dt.float8e3
    u8 = mybir.dt.uint8
    DR = mybir.MatmulPerfMode.DoubleRow

    # All big inputs are partition-major [128, W] in DRAM (one fat row per
    # partition -> one large DMA descriptor per partition line; ~2-4KB rows
    # hit the per-packet sweet spot of the DMA engines).
    cvidx_d = nc.dram_tensor("cvidx", [P, 8 + TOK], u8,
                             kind="ExternalInput").ap()
    xw0_d = nc.dram_tensor("xw0", [P, XW_W], f8e3,
                           kind="ExternalInput").ap()
    xw1_d = nc.dram_tensor("xw1", [P, XW_W], f8e3,
                           kind="ExternalInput").ap()
    xw2_d = nc.dram_tensor("xw2", [P, XW_W], f8e3,
                           kind="ExternalInput").ap()
    leda_d = nc.dram_tensor("leda", [P, KD * LEDA_W], f8,
                            kind="ExternalInput").ap()
    ledb_d = nc.dram_tensor("ledb", [P, KD * LEDB_W], f8,
                            kind="ExternalInput").ap()
    lu_d = nc.dram_tensor("lu8", [P, 2 * DOUT], f8, kind="ExternalInput").ap()
    xw34_d = nc.dram_tensor("xw34", [P, 2 * XW_W], f8e3,
                            kind="ExternalInput").ap()
    xw5_d = nc.dram_tensor("xw5", [P, XW_W], f8e3,
                           kind="ExternalInput").ap()
    out_d = nc.dram_tensor("out", [TOK, DOUT], bf16, kind="ExternalOutput").ap()

    with tile.TileContext(nc) as tc:
        with (
            tc.tile_pool(name="consts", bufs=1) as consts,
            tc.tile_pool(name="outsb", bufs=5) as outsb,
        ):
            # Input DMAs: all doorbells on the sync queue in strict
            # need-order (DMA-ring work order = enqueue order; a transfer's
            # completion ~= serial-prefix-bytes/~265GBps + up to ~1.5us of
            # engine skew).
            # The 16 DMA engines serve ALL enqueued transfers round-robin
            # at ~21.5GB/s each, so an early doorbell steals bandwidth from
            # every in-flight transfer.  Only the critical prefix (cvidx,
            # xw0, xw12 - the wave1 k1 gate) rings immediately; led rings
            # from scalar behind burn-copies that write into the led tiles
            # (WAW ordering the scheduler cannot hoist), and lu8/xw345
            # ring from gpsimd behind memset burns the same way.
            # Per-k-tile transfers: wave1's group pipeline consumes k0 and
            # k1 for the first 6 psum groups while k2 is still streaming,
            # so the dense-stream onset is the xw2 completion (~10.9us).
            # xw0 rings from scalar (earliest body entry, empty Q10 ring)
            # so the k0 groups start ~0.5us sooner.
            xw0 = consts.tile([P, 1, XW_W], f8e3, tag="xw0")
            nc.scalar.dma_start(xw0[:],
                                xw0_d.rearrange("p (k c) -> p k c", c=XW_W))
            xw1 = consts.tile([P, 1, XW_W], f8e3, tag="xw1")
            nc.sync.dma_start(xw1[:],
                              xw1_d.rearrange("p (k c) -> p k c", c=XW_W))
            xw2 = consts.tile([P, 1, XW_W], f8e3, tag="xw2")
            nc.sync.dma_start(xw2[:],
                              xw2_d.rearrange("p (k c) -> p k c", c=XW_W))
            cvidx = consts.tile([P, 8 + TOK], u8, tag="cvidx")
            nc.sync.dma_start(cvidx[:], cvidx_d[:, :])
            cvals = cvidx[:, 0:8].bitcast(f32)
            idx_bc = cvidx[:, 8:8 + TOK]

            # Hoist the scalar engine's one-time ACT_TABLE_LOAD into the
            # boot shadow (it otherwise fires lazily right before the first
            # wave1 copy, delaying it by ~1.5us).
            scratch = consts.tile([1, 8], f32, tag="scratch")
            nc.vector.memset(scratch[:], 0.0)
            nc.scalar.copy(out=scratch[:, 0:4], in_=scratch[:, 4:8])

            zeros = consts.tile([P, 1024], f32, tag="zeros")
            nc.vector.memset(zeros[:], 0.0)

            led_a = consts.tile([P, KD, LEDA_W], f8, tag="led_a")
            led_b = consts.tile([P, KD, LEDB_W], f8, tag="led_b")
            # ~1.4us scalar burn, then the led doorbells (~10.5us, as the
            # critical prefix finishes).
            nc.scalar.copy(out=led_a[:, 0, 0:512], in_=zeros[:, 0:512])
            nc.scalar.copy(out=led_a[:, 1, 0:512], in_=zeros[:, 0:512])
            nc.scalar.dma_start(led_a[:],
                                leda_d.rearrange("p (k c) -> p k c",
                                                 c=LEDA_W))
            nc.scalar.copy(out=led_b[:, 0, 0:336], in_=zeros[:, 0:336])
            nc.scalar.dma_start(led_b[:],
                                ledb_d.rearrange("p (k c) -> p k c",
                                                 c=LEDB_W))

            # PE warmup: dummy matmuls during the boot/DMA shadow covering
            # the full ~3.4us HAM activity window (cold matmuls run at
            # 1.2GHz) AND bridging the PE queue to the xw12 arrival.
            warm = consts.tile([P, 256], bf16, tag="warm")
            nc.gpsimd.memset(warm[:], 0.0)

            lu8 = consts.tile([P, 2, DOUT], f8, tag="lu8")
            xw34 = consts.tile([P, 2, XW_W], f8e3, tag="xw34")
            xw5 = consts.tile([P, 1, XW_W], f8e3, tag="xw5")

            def xt(kk, tsl):
                t_, o = ((xw0, 0) if kk < 1 else
                         (xw1, 1) if kk < 2 else
                         (xw2, 2) if kk < 3 else
                         (xw34, 3) if kk < 5 else (xw5, 5))
                return t_[:, kk - o, tsl]

            def wt(kk, nsl):
                t_, o = ((xw0, 0) if kk < 1 else
                         (xw1, 1) if kk < 2 else
                         (xw2, 2) if kk < 3 else
                         (xw34, 3) if kk < 5 else (xw5, 5))
                return t_[:, kk - o,
                          slice(WOFF + nsl.start, WOFF + nsl.stop)]

            # One-hot masks: mask[p, t] = (concept_idx[t] == cvals[p, jc]).
            # masks on vector (gpsimd runs this op ~20x slower: 9.5us/mask).
            masks = []
            for jc in range(2):
                m = consts.tile([P, TOK], f32, tag=f"mask{jc}")
                nc.vector.tensor_scalar(
                    m[:], idx_bc[:], cvals[:, jc:jc + 1], 1.0,
                    mybir.AluOpType.is_equal, mybir.AluOpType.mult)
                masks.append(m)

            # MT8[(c,r) rows as [part, subtile], t]: routed "down"
            # activations (16*down) in fp8, consumed by the DoubleRow
            # up-matmul.  Rows 200..255 (partitions 72..127 of subtile 1)
            # are zero except the ones row at 224 (partition 96) that
            # contracts with the *1024 bias row of lu8.
            MT8 = consts.tile([P, 2, 640], f8, tag="MT8")
            nc.gpsimd.memset(MT8[64:P, 1, :], 0.0)
            nc.gpsimd.memset(MT8[96:97, 1, :], 0.125)

            # gpsimd burn chain (~1ns/byte-per-partition): doorbells for
            # the wave2 operands fire ~13-15us, after the led stream.
            nc.gpsimd.memset(lu8[:, 0, :], 0.0)
            nc.gpsimd.memset(lu8[:, 1, :], 0.0)
            nc.gpsimd.dma_start(lu8[:],
                                lu_d.rearrange("p (j o) -> p j o", o=DOUT))
            nc.gpsimd.memset(xw34[:, 0, :], 0.0)
            nc.gpsimd.dma_start(xw34[:],
                                xw34_d.rearrange("p (k c) -> p k c", c=XW_W))
            nc.gpsimd.memset(xw5[:, 0, 0:1000], 0.0)
            nc.gpsimd.dma_start(xw5[:],
                                xw5_d.rearrange("p (k c) -> p k c", c=XW_W))

            with (
                tc.tile_pool(name="at_ps", bufs=2, space="PSUM") as at_pool,
                tc.tile_pool(name="out_ps", bufs=6, space="PSUM") as out_pool,
            ):
                # warm matmuls write an out_pool tile; its buffer recycles
                # into a wave1 group long after the warmup retires.
                wps = out_pool.tile([P, 512], f32, tag="ops")
                for i in range(N_WARM):
                    nc.tensor.matmul(wps[:, 0:256], warm[:, 0:P], warm[:, :],
                                     start=(i == 0), stop=(i == N_WARM - 1))

                # A.T[(c,r), t] = (16*lora_down_flat).T @ ed.T for all
                # concepts, 3 DoubleRow matmuls per chunk (contraction row =
                # sub*128 + p).  The led data lands mid-wave1; the scheduler
                # interleaves these chains into the wave1 stream.
                for nh in range(2):
                    for jc in range(2):
                        jp = P if jc == 0 else NJ - P  # 128, 72
                        jsl = slice(jc * P, jc * P + jp)
                        nsl = slice(nh * NH, (nh + 1) * NH)
                        at = at_pool.tile([P, NH], f32, tag="at")
                        for k in range(KD // 2):
                            if nh == 0:
                                rhs = led_a[:, 2 * k:2 * k + 2,
                                            ED_OFF:ED_OFF + NH]
                            else:
                                rhs = led_b[:, 2 * k:2 * k + 2, 0:NH]
                            nc.tensor.matmul(
                                at[:jp, :],
                                led_a[:, 2 * k:2 * k + 2, jsl], rhs,
                                start=(k == 0), stop=(k == KD // 2 - 1),
                                perf_mode=DR)
                        nc.vector.tensor_tensor(
                            MT8[:jp, jc, nsl], at[:jp, :],
                            masks[jc][:jp, nsl], mybir.AluOpType.mult)

                if N_WARM2:
                    # Bridge fillers between the wave1-k0 groups and the
                    # xw12 arrival.
                    wps2 = out_pool.tile([P, 512], f32, tag="ops")
                    for i in range(N_WARM2):
                        nc.tensor.matmul(wps2[:, 0:256], warm[:, 0:P],
                                         warm[:, :], start=(i == 0),
                                         stop=(i == N_WARM2 - 1))

                # Main accumulation: wave1 [org k0..k2] per (t, n) PSUM
                # group -> copy to bf16 osb on scalar; wave2 [upDR, org
                # k3..k5] (DR must open its group: a fp8->DR transition
                # mid-group wedges the PE) -> vector add into osb -> bf16
                # output DMA.
                KA = 3
                osbs = []
                for ti in range(len(T_EDGES) - 1):
                    t0, t1 = T_EDGES[ti], T_EDGES[ti + 1]
                    tw = t1 - t0
                    tsl = slice(t0, t1)
                    osb = outsb.tile([P, DOUT], bf16, tag="osb")
                    osbs.append(osb)
                    for (n0, nw) in N_CHUNKS:
                        nsl = slice(n0, n0 + nw)
                        ps = out_pool.tile([P, 512], f32, tag="ops")
                        for k in range(KA):
                            nc.tensor.matmul(
                                ps[:tw, :nw], xt(k, tsl), wt(k, nsl),
                                start=(k == 0), stop=(k == KA - 1))
                        nc.scalar.copy(out=osb[:tw, nsl], in_=ps[:tw, :nw])
                for ti in range(len(T_EDGES) - 1):
                    t0, t1 = T_EDGES[ti], T_EDGES[ti + 1]
                    tw = t1 - t0
                    tsl = slice(t0, t1)
                    osb = osbs[ti]
                    for (n0, nw) in N_CHUNKS:
                        nsl = slice(n0, n0 + nw)
                        ps = out_pool.tile([P, 512], f32, tag="ops")
                        nc.tensor.matmul(
                            ps[:tw, :nw], MT8[:, 0:2, tsl],
                            lu8[:, 0:2, nsl],
                            start=True, stop=False, perf_mode=DR)
                        for i, k in enumerate(range(KA, KD)):
                            nc.tensor.matmul(
                                ps[:tw, :nw], xt(k, tsl), wt(k, nsl),
                                start=False, stop=(i == KD - KA - 1))
                        nc.vector.tensor_tensor(
                            osb[:tw, nsl], ps[:tw, :nw], osb[:tw, nsl],
                            mybir.AluOpType.add)
                        if ti == len(T_EDGES) - 2:
                            # last t-tile: ship each n-chunk as soon as its
                            # add lands, on the (by now idle) sync input
                            # queue / the gpsimd output queue; scalar's
                            # doorbells measure ~1.5us - avoid it here.
                            eng = (nc.sync, nc.sync, nc.gpsimd)[n0 // 512]
                            eng.dma_start(out_d[tsl, nsl], osb[:tw, nsl])
                    if ti == len(T_EDGES) - 3:
                        # second-to-last t-tile rides the scalar queue
                        # (idle after the wave1 copies; its slow doorbell
                        # is off the critical path here), keeping sync free
                        # to ship the last tile's chunks immediately.
                        nc.scalar.dma_start(out_d[tsl, :], osb[:tw, :])
                    elif ti != len(T_EDGES) - 2:
                        nc.gpsimd.dma_start(out_d[tsl, :], osb[:tw, :])

    nc.compile()
    return nc


def get_bass():
    if "v7" not in _cache:
        _cache["v7"] = _build_bass()
    return _cache["v7"]


def make_in_maps(x, edit_direction, concept_idx, lora_down, lora_up, W, b_lin):
    """Host-side sharding + layout/dtype prep (no reference FLOPs)."""
    f8 = mybir.dt.np(mybir.dt.float8e4)
    f8e3 = mybir.dt.np(mybir.dt.float8e3)
    x = np.asarray(x, dtype=np.float32)
    ed = np.asarray(edit_direction, dtype=np.float32)
    idx = np.asarray(concept_idx)
    ld = np.asarray(lora_down, dtype=np.float32)
    lup = np.asarray(lora_up, dtype=np.float32)
    W = np.asarray(W, dtype=np.float32)
    b = np.asarray(b_lin, dtype=np.float32)

    def pmajor(a, ksub):
        """[ksub*128, W] row-major -> [128, ksub*W] partition-major."""
        w = a.shape[1]
        return np.ascontiguousarray(
            a.reshape(ksub, P, w).transpose(1, 0, 2).reshape(P, ksub * w))

    ldT = ld.transpose(1, 0, 2).reshape(DIN, NJ)                # [768, 200]
    lu8 = np.zeros((2 * P, DOUT), dtype=np.float32)
    lu8[:NJ] = lup.reshape(NJ, DOUT) * 2.0   # x8 (range) x0.25 (alpha/rank)
    lu8[P + 96] = b * 1024.0                 # bias row (*128 psum, ones=1/8)
    lu8 = pmajor(lu8.astype(f8), 2)                             # [128, 2560]
    cv = np.full(2 * P, -1.0, dtype=np.float32)
    cv[:NJ] = np.arange(NJ, dtype=np.float32) // RANK
    cvals = np.ascontiguousarray(cv.reshape(2, P).T)            # [128, 2]
    # Whole org matmul in fp8-e3m4: (2x).T @ (64 W).T accumulates 128*out
    # in PSUM; the host divides the bf16 output by 128 (exact).
    WT8 = np.clip(W.T * 64.0, -15.5, 15.5).astype(f8e3)         # [768, 1280]

    in_maps = []
    for c in range(N_CORES):
        sl = slice(c * BPC, (c + 1) * BPC)
        xs = x[sl].reshape(TOK, DIN)
        eds = ed[sl].reshape(TOK, DIN)
        idxu = idx[sl].reshape(TOK).astype(np.uint8)
        leda = np.zeros((DIN, LEDA_W), dtype=f8)
        leda[:, :NJ] = (ldT * 16.0).astype(f8)
        edT8 = eds.T.astype(f8)
        leda[:, ED_OFF:ED_OFF + NH] = edT8[:, :NH]
        ledb = np.zeros((DIN, LEDB_W), dtype=f8)
        ledb[:, :NH] = edT8[:, NH:]
        cvidx = np.empty((P, 8 + TOK), dtype=np.uint8)
        cvidx[:, 0:8] = cvals.view(np.uint8)
        cvidx[:, 8:] = np.broadcast_to(idxu.reshape(1, TOK), (P, TOK))
        xw = np.zeros((DIN, XW_W), dtype=f8e3)
        xw[:, :TOK] = np.clip(xs.T * 2.0, -15.5, 15.5).astype(f8e3)
        xw[:, WOFF:] = WT8
        xw = pmajor(xw, KD)                                     # [128, 6*XW_W]
        in_maps.append({
            "cvidx": cvidx,
            "xw0": np.ascontiguousarray(xw[:, 0:XW_W]),
            "xw1": np.ascontiguousarray(xw[:, XW_W:2 * XW_W]),
            "xw2": np.ascontiguousarray(xw[:, 2 * XW_W:3 * XW_W]),
            "leda": pmajor(leda, KD),
            "ledb": pmajor(ledb, KD),
            "lu8": lu8,
            "xw34": np.ascontiguousarray(xw[:, 3 * XW_W:5 * XW_W]),
            "xw5": np.ascontiguousarray(xw[:, 5 * XW_W:]),
        })
    return in_maps


def kernel(x, edit_direction, concept_idx, lora_down, lora_up, W, b_lin,
           _trace=False):
    nc = get_bass()
    in_maps = make_in_maps(x, edit_direction, concept_idx, lora_down, lora_up,
                           W, b_lin)
    res = run_bass_kernel_spmd(nc, in_maps, core_ids=list(range(N_CORES)),
                               trace=_trace)
    out = np.concatenate([np.asarray(r["out"], dtype=np.float32)
                          for r in res.results], axis=0) * (1.0 / 128.0)
    out = out.reshape(B, T, DOUT)
    if _trace:
        kernel.last_results = res
    return out
